# revision 25
# baseline (speedup 1.0000x reference)
"""Multi-head attention (SEQ=4096, d_model=1024, 16 heads of d=64) on 8 TRN2
NeuronCores, tensor-parallel over heads (2 heads/core), with per-head
AllToAlls to re-shard from head-parallel to sequence-parallel before the
output projection.

Per core c (heads 2c, 2c+1):
  1. Projections (contraction m=d_model on partitions; host feeds transposed
     bf16 activations qT/kT/vT):  qhT2/khT2 [128(2x64 d), 4096] in SBUF;
     vh natural [ks, dv] per head with a ones column appended (softmax
     denominator comes out of the AV matmul for free).
  2. Attention per head h, per 1024-wide qs chunk: scoresT[ks,qs] =
     khT^T @ qhT (K=d=64); exp via ScalarE LUT / VectorE Schraudolph
     bit-trick (alternating tiles; softmax renormalization cancels the
     bit-trick's sawtooth error); AV in outT orientation:
     avT[65, qs] += vh_aug^T @ PT.  Row 64 = softmax sums.  The UNNORMALIZED
     avT is copied out (dv rows -> outT bf16, sums row -> f32 buffer);
     normalization is deferred to after the AllToAll (it only needs a
     partition-broadcast there, done with one tiny K=2 selector matmul per
     chunk instead of per-(head,chunk) work on the attention critical path).
  3. After head h: AllToAll of its outT ([512,512] bf16, shard j -> core j)
     plus a tiny AllToAll of the sums ([8,512] f32).  Head 0's collectives
     hide under head 1's attention.
  4. FC: load gathered outT_full (dv rows arrive permuted; Wfc is
     pre-permuted on the host to match), scale rows by broadcast
     reciprocal-sums, matmul with full WfcT, relu + residual, write rows
     [512c : 512c+512].  Host concatenates core outputs.
"""

import os
import sys

sys.path.insert(0, "/opt/trn_rl_repo")

import numpy as np
import ml_dtypes

import concourse.bass as bass
import concourse.mybir as mybir
import concourse.tile as tile
from concourse import bacc
from concourse.bass_utils import run_bass_kernel_spmd

# Problem constants (hardcoded per contract)
SEQ = 4096
DM = 1024
NH = 16
DK = 64
DV = 64
CORES = 8
P = 128
HL = 2 * DK  # 128: two heads' head-dim per core
SROWS = SEQ // CORES  # 512 output rows per core
MO = DM // P  # 8 m-chunks of d_model
F32 = mybir.dt.float32
BF16 = mybir.dt.bfloat16

# exp mode: "act" exact LUT | "dve" Schraudolph bit-trick | "split" alternate
EXP_MODE = os.environ.get("EXP_MODE", "split")
EXP_A = 128.0 / float(np.log(2.0))  # bf16-bits Schraudolph slope
EXP_B = 16256.0 - 5.5  # 127*128 - C


def _exp_tile(nc, out_bf16, in_psum, scale, use_dve):
    """out = exp(scale * in), bf16."""
    if use_dve:
        nc.vector.tensor_scalar(
            out=out_bf16.bitcast(mybir.dt.int16),
            in0=in_psum,
            scalar1=float(scale * EXP_A),
            scalar2=float(EXP_B),
            op0=mybir.AluOpType.mult,
            op1=mybir.AluOpType.add,
        )
    else:
        nc.scalar.activation(
            out=out_bf16,
            in_=in_psum,
            func=mybir.ActivationFunctionType.Exp,
            scale=float(scale),
        )


def build(seq=SEQ, exp_mode=None):
    exp_mode = exp_mode or EXP_MODE
    srows = seq // CORES
    kb = seq // P  # key blocks
    qcw = min(512, seq)  # qs chunk width
    qcs = seq // qcw
    qsw = qcw
    nqs = 1
    sb_blocks = srows // P

    nc = bacc.Bacc(
        "TRN2",
        target_bir_lowering=False,
        debug=False,
        enable_asserts=True,
        num_devices=CORES,
    )

    qT = nc.dram_tensor("qT", [DM, seq], BF16, kind="ExternalInput").ap()
    kT = nc.dram_tensor("kT", [DM, seq], BF16, kind="ExternalInput").ap()
    vT = nc.dram_tensor("vT", [DM, seq], BF16, kind="ExternalInput").ap()
    wqT = nc.dram_tensor("wqT", [DM, HL], BF16, kind="ExternalInput").ap()
    wkT = nc.dram_tensor("wkT", [DM, HL], BF16, kind="ExternalInput").ap()
    wvT = nc.dram_tensor("wvT", [DM, HL], BF16, kind="ExternalInput").ap()
    # pre-permuted on host to match the post-A2A dv row order
    wfcT = nc.dram_tensor("wfcT", [DM, DM], BF16, kind="ExternalInput").ap()
    qres = nc.dram_tensor("qres", [srows, DM], F32, kind="ExternalInput").ap()
    out = nc.dram_tensor("out", [srows, DM], F32, kind="ExternalOutput").ap()

    qT_r = qT.rearrange("(o p) s -> p o s", p=P)
    kT_r = kT.rearrange("(o p) s -> p o s", p=P)
    vT_r = vT.rearrange("(o p) s -> p o s", p=P)
    wqT_r = wqT.rearrange("(o p) h -> p o h", p=P)
    wkT_r = wkT.rearrange("(o p) h -> p o h", p=P)
    wvT_r = wvT.rearrange("(o p) h -> p o h", p=P)
    wfcT_r = wfcT.rearrange("(o p) d -> p o d", p=P)
    qres_r = qres.rearrange("(b p) d -> p b d", p=P)
    out_r = out.rearrange("(b p) d -> p b d", p=P)

    with tile.TileContext(nc) as tc:
        with (
            tc.tile_pool(name="const", bufs=1) as cpool,
            tc.tile_pool(name="xin", bufs=8) as xpool,
            tc.tile_pool(name="pt", bufs=5) as ptpool,
            tc.tile_pool(name="small", bufs=2) as spool,
            tc.tile_pool(name="smp", bufs=2) as smpool,
            tc.tile_pool(name="ps", bufs=8, space="PSUM") as ps,
            tc.tile_pool(name="dram", bufs=1, space="DRAM") as dr,
        ):
            # ---- constants / persistent tiles ----
            wq_sb = cpool.tile([P, MO, HL], BF16, tag="wq")
            wk_sb = cpool.tile([P, MO, HL], BF16, tag="wk")
            wv_sb = cpool.tile([P, MO, HL], BF16, tag="wv")
            nc.sync.dma_start(wq_sb[:], wqT_r[:])
            nc.sync.dma_start(wk_sb[:], wkT_r[:])
            nc.sync.dma_start(wv_sb[:], wvT_r[:])

            qhT2 = cpool.tile([P, seq], BF16, tag="qhT2")
            khT2z = [
                cpool.tile([P, seq], BF16, tag=f"khT2z{h}", name=f"khT2z{h}")
                for h in range(2)
            ]
            # ~1e-30 instead of 0.0: zero-valued weights measurably slow the
            # PE (~259 vs 216 ns/mm; value-dependent activity gating).  The
            # padded rows contribute ~1e-30 * O(1) to fp32 scores -- negligible.
            nc.vector.memset(khT2z[0][DK:P, :], 1e-30)
            nc.vector.memset(khT2z[1][0:DK, :], 1e-30)
            vh = [
                cpool.tile([P, kb, DV + 1], BF16, tag=f"vh{h}", name=f"vh{h}")
                for h in range(2)
            ]
            nc.vector.memset(vh[0][:, :, DV : DV + 1], 1.0)
            nc.vector.memset(vh[1][:, :, DV : DV + 1], 1.0)
            outT = cpool.tile([P, seq], BF16, tag="outT")
            sums = [
                cpool.tile([P, min(1024, seq)], F32, tag=f"sums{h}", name=f"sums{h}")
                for h in range(2)
            ]
            nc.vector.memset(sums[0][:], 1.0)
            nc.vector.memset(sums[1][:], 1.0)

            # FC-phase selector constant (host-fed; partition-1 writes are
            # not expressible on-device)
            sel_in = nc.dram_tensor("sel", [2, P], F32, kind="ExternalInput").ap()
            sel = cpool.tile([2, P], F32, tag="sel")

            # ---- phase 1: projections (q, k then v), full-width ----
            def load_chunks(src_r):
                xts = []
                for o in range(MO):
                    xt = xpool.tile([P, seq], BF16, tag="xin", name=f"xin{o}")
                    eng = nc.sync if o % 2 == 0 else nc.gpsimd
                    eng.dma_start(xt[:], src_r[:, o, :])
                    xts.append(xt)
                return xts

            pgroups = seq // 512 if seq >= 512 else 1
            pgw = seq // pgroups  # accumulator width (<= 512)
            for w_sb, src_r, dstT in ((wq_sb, qT_r, qhT2), (wk_sb, kT_r, None)):
                xts = load_chunks(src_r)
                pps = [
                    ps.tile([P, pgw], F32, tag="ps", name=f"pp{g}")
                    for g in range(pgroups)
                ]
                for o in range(MO):
                    for g in range(pgroups):
                        for j in range(pgw // qsw):
                            nc.tensor.matmul(
                                pps[g][:HL, j * qsw : (j + 1) * qsw],
                                w_sb[:, o, :],
                                xts[o][:, g * pgw + j * qsw :][:, :qsw],
                                start=(o == 0),
                                stop=(o == MO - 1),
                            )
                for g in range(pgroups):
                    if dstT is not None:
                        nc.scalar.copy(
                            out=dstT[:, g * pgw : (g + 1) * pgw], in_=pps[g][:HL]
                        )
                    else:
                        nc.scalar.copy(
                            out=khT2z[0][0:DK, g * pgw : (g + 1) * pgw],
                            in_=pps[g][0:DK],
                        )
                        nc.scalar.copy(
                            out=khT2z[1][DK:HL, g * pgw : (g + 1) * pgw],
                            in_=pps[g][DK:HL],
                        )
            xts = load_chunks(vT_r)
            for b in range(kb):
                pv = ps.tile([P, HL], F32, tag="ps")
                for o in range(MO):
                    nc.tensor.matmul(
                        pv[:],
                        xts[o][:, b * P : (b + 1) * P],
                        wv_sb[:, o, :],
                        start=(o == 0),
                        stop=(o == MO - 1),
                    )
                nc.vector.tensor_copy(out=vh[0][:, b, :DV], in_=pv[:, :DK])
                nc.vector.tensor_copy(out=vh[1][:, b, :DV], in_=pv[:, DK:HL])

            # late constants: queued after the projection input DMAs so the
            # first q chunks are not delayed behind the 2MB wfc load
            nc.sync.dma_start(sel[:], sel_in[:])
            wfc_sb = cpool.tile([P, MO, DM], BF16, tag="wfc")
            nc.sync.dma_start(wfc_sb[:], wfcT_r[:])

            # ---- phase 2+3: attention per head, then that head's A2As ----
            a2a_dv_in, a2a_dv_out, a2a_sm_in, a2a_sm_out = [], [], [], []
            hchunks = (CORES * DK) // P  # 4 lhsT chunks per head
            ofull, recips = [None, None], [None, None]

            def _fc_load(h):
                of = cpool.tile([P, hchunks, srows], BF16, tag=f"of{h}", name=f"of{h}")
                nc.sync.dma_start(
                    of[:], a2a_dv_out[h].rearrange("(o p) s -> p o s", p=P)
                )
                rc = smpool.tile([2, hchunks, srows], F32, tag="sm", name=f"sm{h}")
                nc.sync.dma_start(
                    rc[:], a2a_sm_out[h].rearrange("(o i) s -> i o s", i=2)
                )
                ofull[h] = of
                recips[h] = rc

            def _fc_scale(h):
                # normalize gathered dv rows in place (shipped recips)
                for o in range(hchunks):
                    bc = ps.tile([P, srows], F32, tag="ps")
                    nc.tensor.matmul(
                        bc[:], sel[:], recips[h][:, o, :], start=True, stop=True
                    )
                    nc.vector.tensor_mul(
                        out=ofull[h][:, o, :], in0=ofull[h][:, o, :], in1=bc[:]
                    )

            def _fc_prep(h):
                _fc_load(h)
                _fc_scale(h)
            exp_ctr = 0
            for h in range(2):
                hs = h * DK
                for qc in range(qcs):
                    q0 = qc * qcw
                    avT = ps.tile([DV + 1, qcw], F32, tag="ps")
                    for b in range(kb):
                        sco = ps.tile([P, qcw], F32, tag="ps")
                        for j in range(nqs):
                            nc.tensor.matmul(
                                sco[:, j * qsw : (j + 1) * qsw],
                                khT2z[h][:, b * P : (b + 1) * P],
                                qhT2[:, q0 + j * qsw :][:, :qsw],
                                start=True,
                                stop=True,
                            )
                        pt = ptpool.tile([P, qcw], BF16, tag="pt")
                        if exp_mode == "split":
                            # 4/9 of tiles on DVE: DVE carries more non-exp
                            # work, so ACT gets the larger share
                            use_dve = (exp_ctr % 9) in (0, 2, 4, 6)
                        else:
                            use_dve = exp_mode == "dve"
                        exp_ctr += 1
                        _exp_tile(nc, pt[:], sco[:], 1.0 / np.sqrt(DK), use_dve)
                        for j in range(nqs):
                            nc.tensor.matmul(
                                avT[:, j * qsw : (j + 1) * qsw],
                                vh[h][:, b, :],
                                pt[:, j * qsw : (j + 1) * qsw],
                                start=(b == 0),
                                stop=(b == kb - 1),
                            )
                    # drain unnormalized: dv rows on ACT, sums row on DVE
                    nc.scalar.copy(
                        out=outT[hs : hs + DK, q0 : q0 + qcw], in_=avT[:DV, :]
                    )
                    nc.vector.tensor_copy(
                        out=sums[h][
                            32 * (qc // 2) : 32 * (qc // 2) + 1,
                            (qc % 2) * 512 : (qc % 2) * 512 + qcw,
                        ],
                        in_=avT[DV : DV + 1, :],
                    )
                    if h == 1 and qc == max(0, qcs - 2):
                        _fc_prep(0)
                # ship reciprocals: recip commutes with the AllToAll, and this
                # takes the reciprocal off the exposed post-collective chain
                nc.vector.reciprocal(sums[h][:], sums[h][:])
                # head h's collectives (head 0's hide under head 1's compute)
                dvi = dr.tile([CORES * DK, srows], BF16, name=f"a2advi{h}")
                dvo = dr.tile([CORES * DK, srows], BF16, name=f"a2advo{h}")
                smi = dr.tile([CORES, srows], F32, name=f"a2asmi{h}")
                smo = dr.tile([CORES, srows], F32, name=f"a2asmo{h}")
                for j in range(CORES):
                    nc.sync.dma_start(
                        dvi[j * DK : (j + 1) * DK, :],
                        outT[hs : hs + DK, j * srows : (j + 1) * srows],
                    )
                    g0 = j * srows
                    jqc = g0 // 512
                    jcol = (jqc % 2) * 512 + (g0 % 512)
                    nc.sync.dma_start(
                        smi[j : j + 1, :],
                        sums[h][32 * (jqc // 2) : 32 * (jqc // 2) + 1,
                                jcol : jcol + srows],
                    )
                nc.gpsimd.collective_compute(
                    "AllToAll",
                    mybir.AluOpType.bypass,
                    replica_groups=[list(range(CORES))],
                    ins=[dvi.opt()],
                    outs=[dvo.opt()],
                )
                nc.gpsimd.collective_compute(
                    "AllToAll",
                    mybir.AluOpType.bypass,
                    replica_groups=[list(range(CORES))],
                    ins=[smi.opt()],
                    outs=[smo.opt()],
                )
                a2a_dv_in.append(dvi)
                a2a_dv_out.append(dvo)
                a2a_sm_in.append(smi)
                a2a_sm_out.append(smo)

            # ---- phase 4: FC + epilogue ----
            _fc_load(1)
            tiles_fc = [(sb, nm) for sb in range(sb_blocks) for nm in range(DM // 512)]
            qres_sb = {}
            for sb in range(sb_blocks):
                qre = smpool.tile([P, DM], F32, tag="qre", name=f"qre{sb}")
                nc.sync.dma_start(qre[:], qres_r[:, sb, :])
                qres_sb[sb] = qre
            # 3+3+2 grouping: six head-0 accumulation passes run before the
            # head-1 scale point (which must wait on the sums AllToAll), so
            # the PE has ~13us of FC work to chew on during the collective;
            # two PSUM slots stay free for the broadcast matmuls.
            groups = [tiles_fc[0:3], tiles_fc[3:6], tiles_fc[6:8]]
            pfs_all = {}

            def _fc_passA(sb, nm):
                pf = ps.tile([P, 512], F32, tag="ps", name=f"pf{sb}_{nm}")
                pfs_all[(sb, nm)] = pf
                for o in range(hchunks):
                    nc.tensor.matmul(
                        pf[:],
                        ofull[0][:, o, sb * P : (sb + 1) * P],
                        wfc_sb[:, o, nm * 512 : (nm + 1) * 512],
                        start=(o == 0),
                        stop=False,
                    )

            def _fc_passB_epi(sb, nm):
                pf = pfs_all[(sb, nm)]
                for o in range(hchunks):
                    nc.tensor.matmul(
                        pf[:],
                        ofull[1][:, o, sb * P : (sb + 1) * P],
                        wfc_sb[:, hchunks + o, nm * 512 : (nm + 1) * 512],
                        start=False,
                        stop=(o == hchunks - 1),
                    )
                eo = spool.tile([P, 512], F32, tag="eo")
                nc.vector.tensor_scalar_max(out=eo[:], in0=pf[:], scalar1=0.0)
                nc.vector.tensor_add(
                    out=eo[:],
                    in0=eo[:],
                    in1=qres_sb[sb][:, nm * 512 : (nm + 1) * 512],
                )
                nc.sync.dma_start(out_r[:, sb, nm * 512 : (nm + 1) * 512], eo[:])

            for gi in (0, 1):
                for sb, nm in groups[gi]:
                    _fc_passA(sb, nm)
            _fc_scale(1)
            for gi in (0, 1):
                for sb, nm in groups[gi]:
                    _fc_passB_epi(sb, nm)
            for sb, nm in groups[2]:
                _fc_passA(sb, nm)
            for sb, nm in groups[2]:
                _fc_passB_epi(sb, nm)

    nc.compile()
    return nc


def _fc_perm():
    """Row permutation of WfcT matching the post-A2A dv order: FC lhsT chunk
    o (of head-h stream) partition p holds global dv row
    128*(2o + p//64) + h*64 + (p%64)."""
    perm = []
    for h in range(2):
        for o in range(4):
            for p in range(P):
                perm.append(128 * (2 * o + p // 64) + h * 64 + (p % 64))
    return np.array(perm)


def make_in_maps(q, k, v, Wq, Wk, Wv, Wfc, seq=SEQ):
    srows = seq // CORES
    bf = ml_dtypes.bfloat16
    qT = np.ascontiguousarray(q.T).astype(bf)
    kT = np.ascontiguousarray(k.T).astype(bf)
    vT = np.ascontiguousarray(v.T).astype(bf)
    wfcT = np.ascontiguousarray(Wfc.T[_fc_perm()]).astype(bf)
    sel = np.zeros((2, P), np.float32)
    sel[0, :DK] = 1.0
    sel[1, DK:] = 1.0
    in_maps = []
    for c in range(CORES):
        sl = slice(c * HL, (c + 1) * HL)
        in_maps.append(
            {
                "qT": qT,
                "kT": kT,
                "vT": vT,
                "wqT": np.ascontiguousarray(Wq[sl].T).astype(bf),
                "wkT": np.ascontiguousarray(Wk[sl].T).astype(bf),
                "wvT": np.ascontiguousarray(Wv[sl].T).astype(bf),
                "wfcT": wfcT,
                "sel": sel,
                "qres": np.ascontiguousarray(q[c * srows : (c + 1) * srows]).astype(
                    np.float32
                ),
            }
        )
    return in_maps


_NC_CACHE = {}


def kernel(q, k, v, Wq, Wk, Wv, Wfc):
    key = "full"
    if key not in _NC_CACHE:
        _NC_CACHE[key] = build()
    nc = _NC_CACHE[key]
    in_maps = make_in_maps(q, k, v, Wq, Wk, Wv, Wfc)
    trace = bool(int(os.environ.get("KERNEL_TRACE", "0")))
    tc_env = os.environ.get("KERNEL_TRACE_CORES", "")
    kw = {}
    if tc_env:
        kw["trace_cores"] = [int(x) for x in tc_env.split(",")]
    res = run_bass_kernel_spmd(nc, in_maps, list(range(CORES)), trace=trace, **kw)
    if trace:
        kernel.last_exec_time_ns = res.exec_time_ns
        kernel.last_profile = res
    out = np.concatenate([res.results[c]["out"] for c in range(CORES)], axis=0)
    return out.astype(np.float32)
